# revision 10
# baseline (speedup 1.0000x reference)
"""EvaAttention TRN2 kernel: data-parallel over batch across 8 NeuronCores.

Per core (2 batches): qkv proj (fp32r matmuls), per-head QK layernorm + RoPE
(folded into host-precomputed cos/sin tables incl. scale and qn_g), attention
with no-max-subtraction softmax computed entirely in S^T layout (softmax
denominator via ones-augmented V column), scale_norm + proj.
"""
import os
import sys

for _p in (
    "/root/.axon_site",
    "/root/.axon_site/_ro/trn_rl_repo",
    "/root/.axon_site/_ro/pypackages",
    "/opt/trn_rl_repo",
    "/opt/pypackages",
):
    if os.path.isdir(_p) and _p not in sys.path:
        sys.path.append(_p)

import numpy as np

import concourse.bass as bass
import concourse.bacc as bacc
import concourse.tile as tile
from concourse import mybir, masks
from concourse.bass_utils import run_bass_kernel_spmd

F32 = mybir.dt.float32
F32R = mybir.dt.float32r
Act = mybir.ActivationFunctionType
Alu = mybir.AluOpType
X = mybir.AxisListType.X

B, N, C, H, D = 16, 1025, 1024, 16, 64
EPS = 1e-6
SCALE = D ** -0.5
NCORES = 8
BL = B // NCORES          # batches per core
NT = 9                    # token tiles per batch (pad 1025 -> 1152)
NPAD = NT * 128
HH = 2                    # head halves
HPH = H // HH             # heads per half (8)
PAIRS = HPH // 2          # head pairs per half (4)

_CACHE = {}


def _bcast_mid(ap2d, n):
    """[P, F] AP -> [P, n, F] with step-0 middle dim (free-dim broadcast)."""
    return bass.AP(tensor=ap2d.tensor, offset=ap2d.offset,
                   ap=[ap2d.ap[0], [0, n], ap2d.ap[1]])


def _build(has_kbias, has_pbias, repeat=1):
    nc = bacc.Bacc("TRN2", target_bir_lowering=False, debug=False,
                   num_devices=NCORES)

    x_in = nc.dram_tensor("x", [BL, N, C], F32R, kind="ExternalInput").ap()
    wt = nc.dram_tensor("wt", [C, 3 * C], F32R, kind="ExternalInput").ap()
    qkvb = nc.dram_tensor("qkvb", [3 * C], F32R, kind="ExternalInput").ap()
    ropet = nc.dram_tensor("ropet", [4, NPAD, D], F32R, kind="ExternalInput").ap()
    pwt = nc.dram_tensor("pwt", [C, C], F32R, kind="ExternalInput").ap()
    pbias = nc.dram_tensor("pbias", [C], F32R, kind="ExternalInput").ap()
    ident_d = nc.dram_tensor("ident", [128, 128], F32R,
                             kind="ExternalInput").ap()
    onesd = nc.dram_tensor("onesd", [1], F32R, kind="ExternalInput").ap()
    y = nc.dram_tensor("y", [BL, N, C], F32, kind="ExternalOutput").ap()

    with tile.TileContext(nc) as tc:
        with tc.tile_pool(name="consts", bufs=1) as consts:
            ident = consts.tile([128, 128], F32R)
            nc.sync.dma_start(out=ident, in_=ident_d)
            epst = consts.tile([128, 1], F32)
            nc.vector.memset(epst, EPS)
            # rope tables: [0]=cos_q [1]=sin_q [2]=cos_k [3]=sin_k
            rtab = consts.tile([128, 4, NT, D], F32R)
            nc.sync.dma_start(
                out=rtab, in_=ropet.rearrange("f (t p) d -> p f t d", p=128))
            biasb = consts.tile([128, 3 * C], F32R)
            nc.sync.dma_start(
                out=biasb,
                in_=bass.AP(tensor=qkvb.tensor, offset=qkvb.offset,
                            ap=[[0, 128], [1, 3 * C]]))
            if has_pbias:
                pbb = consts.tile([128, C], F32R)
                nc.sync.dma_start(
                    out=pbb,
                    in_=bass.AP(tensor=pbias.tensor, offset=pbias.offset,
                                ap=[[0, 128], [1, C]]))

            nc._epst = epst
            nc._onesd = onesd
            for _rep in range(repeat):
                for b in range(BL):
                    _batch(nc, tc, b, x_in, wt, pwt, y, ident, rtab,
                           biasb, pbb if has_pbias else None, has_kbias)
    nc.compile()
    return nc


def _batch(nc, tc, b, x_in, wt, pwt, y, ident, rtab, biasb, pbb,
           has_kbias):
    with tc.tile_pool(name="adram", bufs=1, space="DRAM") as adp:
        attn_t = adp.tile([NPAD, C], F32R)
        with tc.tile_pool(name="xt", bufs=1) as xtp:
            xT = xtp.tile([128, 8, NPAD], F32R)
            _build_xt(nc, tc, b, x_in, ident, xT)
            for hh in range(HH):
                with tc.tile_pool(name="qkt", bufs=1) as qktp:
                    QT = qktp.tile([128, PAIRS, NPAD], F32R)
                    KT = qktp.tile([128, PAIRS, NPAD], F32R)
                    V = qktp.tile([128, NT, HPH, D + 1], F32R)
                    _qkv_half(nc, tc, b, hh, wt, xT, ident, rtab, biasb, V,
                              QT, KT, has_kbias)
                    _attn_half(nc, tc, b, hh, QT, KT, V, ident, attn_t)
        _norm_proj(nc, tc, b, pwt, attn_t, ident, y, pbb)


def _build_xt(nc, tc, b, x_in, ident, xT):
    """Load x[b] and PE-transpose into xT [128c, 8k, NPAD tok]."""
    with (
        tc.tile_pool(name="xraw", bufs=3) as xrp,
        tc.tile_pool(name="xps", bufs=2, space="PSUM") as xps,
    ):
        for t in range(NT):
            xraw = xrp.tile([128, C], F32R)
            rows = 128 if t < NT - 1 else N - 128 * (NT - 1)
            nc.sync.dma_start(out=xraw[:rows, :],
                              in_=x_in[b, t * 128:t * 128 + rows, :])
            for k in range(8):
                ps = xps.tile([128, 128], F32R)
                nc.tensor.transpose(ps, xraw[:, k * 128:(k + 1) * 128],
                                    ident[:])
                if k % 2 == 0:
                    nc.scalar.copy(out=xT[:, k, t * 128:(t + 1) * 128],
                                   in_=ps)
                else:
                    nc.vector.tensor_copy(
                        out=xT[:, k, t * 128:(t + 1) * 128], in_=ps)


def _qkv_half(nc, tc, b, hh, wt, xT, ident, rtab, biasb, V, QT, KT,
              has_kbias):
    """qkv matmuls for one head-half + LN + RoPE + transposes into QT/KT/V."""
    with (
        tc.tile_pool(name="wp", bufs=2) as wp,
        tc.tile_pool(name="qps", bufs=3, space="PSUM") as qps,
        tc.tile_pool(name="prep", bufs=2) as prep,
        tc.tile_pool(name="stat", bufs=4) as stp,
        tc.tile_pool(name="trps", bufs=2, space="PSUM") as trps,
    ):
        # ones column of V (col D); untouched pad rows are never read
        nc.sync.dma_start(
            out=V[:, :, :, D:D + 1].rearrange("p t h o -> p (t h) o"),
            in_=bass.AP(tensor=nc._onesd.tensor, offset=nc._onesd.offset,
                        ap=[[0, 128], [0, NT * HPH], [1, 1]]))
        for oc in range(3):  # 0=q cols, 1=k cols, 2=v cols
            col0 = hh * 1536 + oc * 512
            wchunk = wp.tile([128, 8, 512], F32R)
            nc.sync.dma_start(
                out=wchunk,
                in_=wt[:, col0:col0 + 512].rearrange("(k p) o -> p k o",
                                                     p=128))
            for t in range(NT):
                ps = qps.tile([128, 512], F32)
                for k in range(8):
                    nc.tensor.matmul(ps, xT[:, k, t * 128:(t + 1) * 128],
                                     wchunk[:, k, :], start=(k == 0),
                                     stop=(k == 7))
                if oc == 2:
                    # v: bias add straight into V tile
                    nc.vector.tensor_tensor(
                        out=V[:, t, :, 0:D],
                        in0=ps.rearrange("p (h d) -> p h d", h=HPH),
                        in1=biasb[:, col0:col0 + 512].rearrange(
                            "p (h d) -> p h d", h=HPH),
                        op=Alu.add)
                    continue
                raw = prep.tile([128, HPH, D], F32R, tag="raw")
                if oc == 1 and not has_kbias:
                    nc.scalar.copy(out=raw.rearrange("p h d -> p (h d)"),
                                   in_=ps)
                else:
                    nc.vector.tensor_tensor(
                        out=raw.rearrange("p h d -> p (h d)"), in0=ps,
                        in1=biasb[:, col0:col0 + 512], op=Alu.add)
                # ---- stats: mu, rstd per head ----
                sums = stp.tile([128, HPH], F32, tag="sums")
                nc.vector.tensor_reduce(sums, raw, axis=X, op=Alu.add)
                sq = prep.tile([128, HPH * D], F32, tag="sq")
                nc.scalar.square(sq, raw.rearrange("p h d -> p (h d)"))
                s2 = stp.tile([128, HPH], F32, tag="s2")
                nc.vector.tensor_reduce(
                    s2, sq.rearrange("p (h d) -> p h d", h=HPH), axis=X,
                    op=Alu.add)
                mu = stp.tile([128, HPH], F32, tag="mu")
                nc.vector.tensor_scalar(mu, sums, 1.0 / D, None, op0=Alu.mult)
                var = stp.tile([128, HPH], F32, tag="var")
                # var = s2/D - mu^2  (computed as (s2*(1/D) - mu*mu))
                nc.vector.tensor_tensor(out=var, in0=mu, in1=mu, op=Alu.mult)
                nc.vector.tensor_scalar(s2, s2, 1.0 / D, None, op0=Alu.mult)
                nc.vector.tensor_tensor(out=var, in0=s2, in1=var,
                                        op=Alu.subtract)
                sd = stp.tile([128, HPH], F32, tag="sd")
                nc.scalar.activation(sd, var, Act.Sqrt, bias=nc._epst[:, 0:1])
                rstd = stp.tile([128, HPH], F32, tag="rstd")
                nc.vector.reciprocal(rstd, sd)
                # ---- LN apply (gpsimd) ----
                ln = prep.tile([128, HPH, D], F32R, tag="ln")
                for h in range(HPH):
                    nc.gpsimd.tensor_scalar(
                        ln[:, h, :], raw[:, h, :], mu[:, h:h + 1],
                        rstd[:, h:h + 1], op0=Alu.subtract, op1=Alu.mult)
                # ---- RoPE: out = ln*COS + swap(ln)*SIN ----
                ctab = rtab[:, 2 * oc, t, :]      # cos_q or cos_k
                stab = rtab[:, 2 * oc + 1, t, :]  # sin_q or sin_k
                ra = prep.tile([128, HPH, D], F32R, tag="ra")
                nc.vector.tensor_tensor(out=ra, in0=ln,
                                        in1=_bcast_mid(ctab, HPH),
                                        op=Alu.mult)
                rb = prep.tile([128, HPH, D], F32R, tag="rb")
                half = D // 2
                nc.vector.tensor_tensor(
                    out=rb[:, :, 0:half], in0=ln[:, :, half:D],
                    in1=_bcast_mid(stab[:, 0:half], HPH), op=Alu.mult)
                nc.vector.tensor_tensor(
                    out=rb[:, :, half:D], in0=ln[:, :, 0:half],
                    in1=_bcast_mid(stab[:, half:D], HPH), op=Alu.mult)
                rot = prep.tile([128, HPH, D], F32R, tag="rot")
                nc.gpsimd.tensor_tensor(out=rot, in0=ra, in1=rb, op=Alu.add)
                # ---- transpose head pairs into QT/KT ----
                dst = QT if oc == 0 else KT
                for p in range(PAIRS):
                    tp = trps.tile([128, 128], F32R)
                    nc.tensor.transpose(
                        tp, rot.rearrange("p h d -> p (h d)")[:, p * 128:(p + 1) * 128],
                        ident[:])
                    if p % 2 == 0:
                        nc.scalar.copy(out=dst[:, p, t * 128:(t + 1) * 128],
                                       in_=tp)
                    else:
                        nc.vector.tensor_copy(
                            out=dst[:, p, t * 128:(t + 1) * 128], in_=tp)


def _attn_half(nc, tc, b, hh, QT, KT, V, ident, attn_t):
    """Attention for 8 heads of one half; writes attn_t[:, cols]."""
    with (
        tc.tile_pool(name="sps", bufs=3, space="PSUM") as sps,
        tc.tile_pool(name="s8ps", bufs=1, space="PSUM") as s8ps,
        tc.tile_pool(name="pvps", bufs=2, space="PSUM") as pvps,
        tc.tile_pool(name="trp2", bufs=1, space="PSUM") as trp2,
        tc.tile_pool(name="pt", bufs=4) as ptp,
        tc.tile_pool(name="att", bufs=3) as attp,
    ):
        for hl in range(HPH):
            hg = hh * HPH + hl
            pr, r = hl // 2, 64 * (hl % 2)

            def kt_l(kt):
                return KT[r:r + 64, pr, kt * 128:(kt + 1) * 128]

            kstr = KT[r:r + 64, pr, 1024:1025]  # straggler kv token 1024
            for qc in range(2):  # q-token chunks of 512
                qs = QT[r:r + 64, pr, qc * 512:(qc + 1) * 512]
                pv = pvps.tile([D + 1, 512], F32, tag="pv")
                for kt in range(8):
                    sp = sps.tile([128, 512], F32, tag="sp")
                    nc.tensor.matmul(sp, kt_l(kt), qs)
                    pt = ptp.tile([128, 512], F32R, tag="pt")
                    nc.scalar.activation(pt, sp[:], Act.Exp)
                    nc.tensor.matmul(pv, V[:, kt, hl, :], pt,
                                     start=(kt == 0), stop=False)
                s8 = s8ps.tile([1, 512], F32, tag="s8")
                nc.tensor.matmul(s8, kstr, qs)
                pt8 = ptp.tile([1, 512], F32R, tag="pt8")
                nc.scalar.activation(pt8, s8[:], Act.Exp)
                nc.tensor.matmul(pv, V[0:1, 8, hl, :], pt8, start=False,
                                 stop=True)
                pvs = attp.tile([D + 1, 512], F32R, tag="pvs")
                nc.vector.tensor_copy(out=pvs, in_=pv)
                for j in range(4):
                    tr = trp2.tile([128, D + 1], F32, tag="tr")
                    nc.tensor.transpose(
                        tr, pvs[:, j * 128:(j + 1) * 128].bitcast(F32),
                        ident[0:D + 1, 0:D + 1].bitcast(F32))
                    rl = attp.tile([128, 1], F32, tag="rl")
                    nc.vector.reciprocal(rl, tr[:, D:D + 1])
                    stage = attp.tile([128, D], F32R, tag="stage")
                    nc.scalar.activation(stage, tr[:, 0:D], Act.Copy,
                                         scale=rl)
                    tok = qc * 512 + j * 128
                    nc.sync.dma_start(
                        out=attn_t[tok:tok + 128, hg * D:(hg + 1) * D],
                        in_=stage)
            # ---- straggler q tokens 1023:1025 (N=2; token 1023 redone) ----
            qstr = QT[r:r + 64, pr, 1023:1025]
            sp1 = s8ps.tile([128, 18], F32, tag="sp1")
            for kt in range(8):
                nc.tensor.matmul(sp1[:, 2 * kt:2 * kt + 2], kt_l(kt), qstr)
            nc.tensor.matmul(sp1[0:1, 16:18], kstr, qstr)
            p1 = ptp.tile([128, 18], F32R, tag="p1")
            nc.scalar.activation(p1, sp1[:], Act.Exp)
            pv1 = pvps.tile([D + 1, 2], F32, tag="pv")
            for kt in range(8):
                nc.tensor.matmul(pv1, V[:, kt, hl, :],
                                 p1[:, 2 * kt:2 * kt + 2],
                                 start=(kt == 0), stop=False)
            nc.tensor.matmul(pv1, V[0:1, 8, hl, :], p1[0:1, 16:18],
                             start=False, stop=True)
            pvs1 = attp.tile([D + 1, 2], F32R, tag="pvs")
            nc.vector.tensor_copy(out=pvs1, in_=pv1)
            tr1 = trp2.tile([128, D + 1], F32, tag="tr")
            nc.tensor.transpose(tr1[0:2, :], pvs1.bitcast(F32),
                                ident[0:D + 1, 0:D + 1].bitcast(F32))
            rl1 = attp.tile([128, 1], F32, tag="rl")
            nc.vector.reciprocal(rl1[0:2, :], tr1[0:2, D:D + 1])
            stage1 = attp.tile([128, D], F32R, tag="stage")
            nc.scalar.activation(stage1[0:2, :], tr1[0:2, 0:D], Act.Copy,
                                 scale=rl1[0:2, :])
            nc.sync.dma_start(out=attn_t[1023:1025, hg * D:(hg + 1) * D],
                              in_=stage1[0:2, :])


def _norm_proj(nc, tc, b, pwt, attn_t, ident, y, pbb):
    """scale_norm over C + proj matmul + output DMA for batch b."""
    with (
        tc.tile_pool(name="lnt", bufs=1) as lntp,
        tc.tile_pool(name="ain", bufs=2) as ainp,
        tc.tile_pool(name="lst", bufs=4) as lstp,
        tc.tile_pool(name="lps", bufs=2, space="PSUM") as lps,
    ):
        lnT = lntp.tile([128, 8, NPAD], F32R)
        for t in range(NT):
            rows = 128 if t < NT - 1 else N - 128 * (NT - 1)
            a = ainp.tile([128, C], F32R, tag="a")
            nc.sync.dma_start(out=a[:rows, :],
                              in_=attn_t[t * 128:t * 128 + rows, :])
            s = lstp.tile([128, 1], F32, tag="s")
            nc.vector.tensor_reduce(s, a, axis=X, op=Alu.add)
            sq = ainp.tile([128, C], F32, tag="lsq")
            nc.scalar.square(sq, a)
            s2 = lstp.tile([128, 1], F32, tag="ls2")
            nc.vector.tensor_reduce(s2, sq, axis=X, op=Alu.add)
            mu = lstp.tile([128, 1], F32, tag="lmu")
            nc.vector.tensor_scalar(mu, s, 1.0 / C, None, op0=Alu.mult)
            var = lstp.tile([128, 1], F32, tag="lvar")
            nc.vector.tensor_tensor(out=var, in0=mu, in1=mu, op=Alu.mult)
            nc.vector.tensor_scalar(s2, s2, 1.0 / C, None, op0=Alu.mult)
            nc.vector.tensor_tensor(out=var, in0=s2, in1=var,
                                    op=Alu.subtract)
            sd = lstp.tile([128, 1], F32, tag="lsd")
            nc.scalar.activation(sd, var, Act.Sqrt, bias=nc._epst[:, 0:1])
            rstd = lstp.tile([128, 1], F32, tag="lrstd")
            nc.vector.reciprocal(rstd, sd)
            ln = ainp.tile([128, C], F32R, tag="ln2")
            nc.vector.tensor_scalar(ln, a, mu[:, 0:1], rstd[:, 0:1],
                                    op0=Alu.subtract, op1=Alu.mult)
            with tc.tile_pool(name="ltps", bufs=2, space="PSUM") as ltps:
                for k in range(8):
                    tp = ltps.tile([128, 128], F32R)
                    nc.tensor.transpose(tp, ln[:, k * 128:(k + 1) * 128],
                                        ident[:])
                    if k % 2 == 0:
                        nc.scalar.copy(out=lnT[:, k, t * 128:(t + 1) * 128],
                                       in_=tp)
                    else:
                        nc.vector.tensor_copy(
                            out=lnT[:, k, t * 128:(t + 1) * 128], in_=tp)
        with tc.tile_pool(name="pwp", bufs=2) as pwp:
            for oc in range(2):
                wchunk = pwp.tile([128, 8, 512], F32R)
                nc.sync.dma_start(
                    out=wchunk,
                    in_=pwt[:, oc * 512:(oc + 1) * 512].rearrange(
                        "(k p) o -> p k o", p=128))
                for t in range(NT):
                    ps = lps.tile([128, 512], F32)
                    for k in range(8):
                        nc.tensor.matmul(ps, lnT[:, k, t * 128:(t + 1) * 128],
                                         wchunk[:, k, :], start=(k == 0),
                                         stop=(k == 7))
                    ostage = ainp.tile([128, 512], F32, tag="ostage")
                    if pbb is not None:
                        nc.vector.tensor_tensor(
                            out=ostage, in0=ps,
                            in1=pbb[:, oc * 512:(oc + 1) * 512], op=Alu.add)
                    else:
                        nc.scalar.copy(out=ostage, in_=ps)
                    rows = 128 if t < NT - 1 else N - 128 * (NT - 1)
                    nc.sync.dma_start(
                        out=y[b, t * 128:t * 128 + rows,
                              oc * 512:(oc + 1) * 512],
                        in_=ostage[:rows, :])


def _host_prep(inputs):
    """Precompute permuted/transposed weights and folded rope tables."""
    perm = np.concatenate([np.arange(0, D, 2), np.arange(1, D, 2)])
    swap = np.concatenate([np.arange(D // 2, D), np.arange(0, D // 2)])

    qkv_w = np.asarray(inputs["qkv_w"], np.float32)
    rope = np.asarray(inputs["rope"], np.float32)
    sin_t, cos_t = rope[:, :D], rope[:, D:]

    # column order: [half][q|k|v][head-in-half][d]  (d permuted for q,k)
    row_order = np.empty(3 * C, np.int64)
    col = 0
    for hh in range(HH):
        for grp in range(3):
            for h in range(hh * HPH, (hh + 1) * HPH):
                base = grp * C + h * D
                idx = base + (perm if grp < 2 else np.arange(D))
                row_order[col:col + D] = idx
                col += D
    wt = np.ascontiguousarray(qkv_w[row_order, :].T)  # [C, 3C]

    qb = np.asarray(inputs["q_bias"], np.float32)
    kb = np.asarray(inputs["k_bias"], np.float32)
    vb = np.asarray(inputs["v_bias"], np.float32)
    full_bias = np.concatenate([qb, kb, vb])
    qkvb = full_bias[row_order].astype(np.float32)

    def make_tables(g, scale):
        gp = np.asarray(g, np.float32)[perm]          # g in permuted coords
        cos_p = cos_t[:, perm]                        # [1024, D]
        sin_p = sin_t[:, perm]
        sgn = np.where(np.arange(D) < D // 2, -1.0, 1.0).astype(np.float32)
        cost = np.zeros((NPAD, D), np.float32)
        sint = np.zeros((NPAD, D), np.float32)
        cost[0] = gp * scale
        cost[1:N] = cos_p * gp[None, :] * scale
        sint[1:N] = sin_p * sgn[None, :] * gp[swap][None, :] * scale
        return cost, sint

    cq, sq_ = make_tables(inputs["qn_g"], SCALE)
    ck, sk = make_tables(inputs["kn_g"], 1.0)
    ropet = np.stack([cq, sq_, ck, sk])  # [4, NPAD, D]

    norm_g = np.asarray(inputs["norm_g"], np.float32)
    norm_b = np.asarray(inputs["norm_b"], np.float32)
    proj_w = np.asarray(inputs["proj_w"], np.float32)
    proj_b = np.asarray(inputs["proj_b"], np.float32)
    pwt = np.ascontiguousarray((proj_w * norm_g[None, :]).T)  # [C, C]
    pbias = (proj_b + norm_b @ proj_w.T).astype(np.float32)

    return wt, qkvb, ropet, pwt, pbias


def kernel(**inputs):
    qn_b = np.asarray(inputs["qn_b"], np.float32)
    kn_b = np.asarray(inputs["kn_b"], np.float32)
    assert not qn_b.any() and not kn_b.any(), \
        "kernel specialized for qn_b == kn_b == 0"

    wt, qkvb, ropet, pwt, pbias = _host_prep(inputs)
    has_kbias = bool(np.asarray(inputs["k_bias"]).any())
    has_pbias = bool(pbias.any())

    key = (has_kbias, has_pbias)
    if key not in _CACHE:
        _CACHE[key] = _build(has_kbias, has_pbias)
    nc = _CACHE[key]

    x = np.asarray(inputs["x"], np.float32)
    in_maps = []
    for c in range(NCORES):
        in_maps.append({
            "x": np.ascontiguousarray(x[c * BL:(c + 1) * BL]),
            "wt": wt, "qkvb": qkvb, "ropet": ropet, "pwt": pwt,
            "pbias": pbias, "ident": np.eye(128, dtype=np.float32),
            "onesd": np.ones(1, dtype=np.float32),
        })
    res = run_bass_kernel_spmd(nc, in_maps, core_ids=list(range(NCORES)))
    out = np.concatenate([res.results[c]["y"] for c in range(NCORES)], axis=0)
    return out.astype(np.float32)


# revision 21
# speedup vs baseline: 1.6611x; 1.6611x over previous
"""EvaAttention TRN2 kernel: data-parallel over batch across 8 NeuronCores.

Per core (2 batches): qkv proj (fp32r matmuls), per-head QK layernorm + RoPE
(folded into host-precomputed cos/sin tables incl. scale and qn_g), attention
with no-max-subtraction softmax computed entirely in S^T layout (softmax
denominator via ones-augmented V column), scale_norm + proj.
"""
import os
import sys

for _p in (
    "/root/.axon_site",
    "/root/.axon_site/_ro/trn_rl_repo",
    "/root/.axon_site/_ro/pypackages",
    "/opt/trn_rl_repo",
    "/opt/pypackages",
):
    if os.path.isdir(_p) and _p not in sys.path:
        sys.path.append(_p)

import numpy as np

import concourse.bass as bass
import concourse.bacc as bacc
import concourse.tile as tile
from concourse import mybir, masks
from concourse.bass_utils import run_bass_kernel_spmd

F32 = mybir.dt.float32
F32R = mybir.dt.float32r
Act = mybir.ActivationFunctionType
Alu = mybir.AluOpType
X = mybir.AxisListType.X

B, N, C, H, D = 16, 1025, 1024, 16, 64
EPS = 1e-6
SCALE = D ** -0.5
NCORES = 8
BL = B // NCORES          # batches per core
NT = 9                    # token tiles per batch (pad 1025 -> 1152)
NPAD = NT * 128
HH = 2                    # head halves
HPH = H // HH             # heads per half (8)
PAIRS = HPH // 2          # head pairs per half (4)

_CACHE = {}


def _bcast_mid(ap2d, n):
    """[P, F] AP -> [P, n, F] with step-0 middle dim (free-dim broadcast)."""
    return bass.AP(tensor=ap2d.tensor, offset=ap2d.offset,
                   ap=[ap2d.ap[0], [0, n], ap2d.ap[1]])


def _build(has_kbias, has_pbias, repeat=1):
    nc = bacc.Bacc("TRN2", target_bir_lowering=False, debug=False,
                   num_devices=NCORES)

    x_in = nc.dram_tensor("x", [BL, N, C], F32R, kind="ExternalInput").ap()
    wt = nc.dram_tensor("wt", [C, 3 * C], F32R, kind="ExternalInput").ap()
    qkvb = nc.dram_tensor("qkvb", [3 * C], F32R, kind="ExternalInput").ap()
    ropet = nc.dram_tensor("ropet", [4, NPAD, D], F32R, kind="ExternalInput").ap()
    pwt = nc.dram_tensor("pwt", [C, C], F32R, kind="ExternalInput").ap()
    pbias = nc.dram_tensor("pbias", [C], F32R, kind="ExternalInput").ap()
    ident_d = nc.dram_tensor("ident", [128, 128], F32R,
                             kind="ExternalInput").ap()
    onesd = nc.dram_tensor("onesd", [1], F32R, kind="ExternalInput").ap()
    y = nc.dram_tensor("y", [BL, N, C], F32, kind="ExternalOutput").ap()

    with tile.TileContext(nc, pool_alloc_mode="queue") as tc:
        with tc.tile_pool(name="consts", bufs=1) as consts:
            ident = consts.tile([128, 128], F32R)
            nc.sync.dma_start(out=ident, in_=ident_d)
            epst = consts.tile([128, 1], F32)
            nc.vector.memset(epst, EPS)
            # rope tables: [0]=cos_q [1]=sin_q [2]=cos_k [3]=sin_k
            rtab = consts.tile([128, 4, NT, D], F32R)
            nc.sync.dma_start(
                out=rtab, in_=ropet.rearrange("f (t p) d -> p f t d", p=128))
            biasb = consts.tile([128, 3 * C], F32R)
            nc.sync.dma_start(
                out=biasb,
                in_=bass.AP(tensor=qkvb.tensor, offset=qkvb.offset,
                            ap=[[0, 128], [1, 3 * C]]))
            if has_pbias:
                pbb = consts.tile([128, C], F32R)
                nc.sync.dma_start(
                    out=pbb,
                    in_=bass.AP(tensor=pbias.tensor, offset=pbias.offset,
                                ap=[[0, 128], [1, C]]))

            nc._epst = epst
            nc._onesd = onesd
            import contextlib
            _psctx = contextlib.ExitStack()
            nc._mmps = _psctx.enter_context(
                tc.tile_pool(name="mmps", bufs=4, space="PSUM"))
            nc._sps = nc._mmps
            nc._pvps = _psctx.enter_context(
                tc.tile_pool(name="pvpsg", bufs=1, space="PSUM"))
            nc._trp = _psctx.enter_context(
                tc.tile_pool(name="trpg", bufs=2, space="PSUM"))
            for _rep in range(repeat):
                for b in range(BL):
                    _batch(nc, tc, b, x_in, wt, pwt, y, ident, rtab,
                           biasb, pbb if has_pbias else None, has_kbias)
            _psctx.close()
    nc.compile()
    return nc


def _batch(nc, tc, b, x_in, wt, pwt, y, ident, rtab, biasb, pbb,
           has_kbias):
    with tc.tile_pool(name="adram", bufs=1, space="DRAM") as adp:
        attn_t = adp.tile([NPAD, C], F32R)
        with tc.tile_pool(name="xt", bufs=1) as xtp:
            xT = [xtp.tile([128, NPAD], F32R, tag=f"xt{k}", name=f"xt{k}")
                  for k in range(8)]
            _build_xt(nc, tc, b, x_in, ident, xT)
            for hh in range(HH):
                with tc.tile_pool(name="qkt", bufs=1) as qktp:
                    QT = qktp.tile([128, PAIRS, NPAD], F32R)
                    KT = qktp.tile([128, PAIRS, NPAD], F32R)
                    V = qktp.tile([128, NT, HPH, D + 1], F32R)
                    _qkv_half(nc, tc, b, hh, wt, xT, ident, rtab, biasb, V,
                              QT, KT, has_kbias)
                    _attn_half(nc, tc, b, hh, QT, KT, V, ident, attn_t)
        _norm_proj(nc, tc, b, pwt, attn_t, ident, y, pbb)


def _build_xt(nc, tc, b, x_in, ident, xT):
    """Load x[b] and PE-transpose into xT [128c, 8k, NPAD tok]."""
    with tc.tile_pool(name="xraw", bufs=3) as xrp:
        xps = nc._trp
        for t in range(NT):
            xraw = xrp.tile([128, C], F32R)
            rows = 128 if t < NT - 1 else N - 128 * (NT - 1)
            nc.sync.dma_start(out=xraw[:rows, :],
                              in_=x_in[b, t * 128:t * 128 + rows, :])
            for k in range(8):
                ps = xps.tile([128, 128], F32R, tag="tr", name="xtr")
                nc.tensor.transpose(ps, xraw[:, k * 128:(k + 1) * 128],
                                    ident[:])
                if k % 2 == 0:
                    nc.scalar.copy(out=xT[k][:, t * 128:(t + 1) * 128],
                                   in_=ps)
                else:
                    nc.vector.tensor_copy(
                        out=xT[k][:, t * 128:(t + 1) * 128], in_=ps)


def _qkv_half(nc, tc, b, hh, wt, xT, ident, rtab, biasb, V, QT, KT,
              has_kbias):
    """qkv matmuls for one head-half + LN + RoPE + transposes into QT/KT/V."""
    with (
        tc.tile_pool(name="wp", bufs=2) as wp,
        tc.tile_pool(name="prep", bufs=2) as prep,
        tc.tile_pool(name="stat", bufs=4) as stp,
    ):
        qps, trps = nc._mmps, nc._trp
        # ones column of V (col D); untouched pad rows are never read
        nc.sync.dma_start(
            out=V[:, :, :, D:D + 1].rearrange("p t h o -> p (t h) o"),
            in_=bass.AP(tensor=nc._onesd.tensor, offset=nc._onesd.offset,
                        ap=[[0, 128], [0, NT * HPH], [1, 1]]))
        for oc in range(3):  # 0=q cols, 1=k cols, 2=v cols
            col0 = hh * 1536 + oc * 512
            wchunk = wp.tile([128, 8, 512], F32R)
            nc.sync.dma_start(
                out=wchunk,
                in_=wt[:, col0:col0 + 512].rearrange("(k p) o -> p k o",
                                                     p=128))
            for t in range(NT):
                ps = qps.tile([128, 512], F32, tag="mm512", name="qkvps")
                for k in range(8):
                    nc.tensor.matmul(ps, xT[k][:, t * 128:(t + 1) * 128],
                                     wchunk[:, k, :], start=(k == 0),
                                     stop=(k == 7))
                if oc == 2:
                    # v: bias add straight into V tile
                    nc.vector.tensor_tensor(
                        out=V[:, t, :, 0:D],
                        in0=ps.rearrange("p (h d) -> p h d", h=HPH),
                        in1=biasb[:, col0:col0 + 512].rearrange(
                            "p (h d) -> p h d", h=HPH),
                        op=Alu.add)
                    continue
                raw = prep.tile([128, HPH, D], F32R, tag="raw")
                if oc == 1 and not has_kbias:
                    nc.scalar.copy(out=raw.rearrange("p h d -> p (h d)"),
                                   in_=ps)
                else:
                    nc.vector.tensor_tensor(
                        out=raw.rearrange("p h d -> p (h d)"), in0=ps,
                        in1=biasb[:, col0:col0 + 512], op=Alu.add)
                # ---- stats: mu, rstd per head ----
                sums = stp.tile([128, HPH], F32, tag="sums")
                nc.vector.tensor_reduce(sums, raw, axis=X, op=Alu.add)
                sq = prep.tile([128, HPH * D], F32, tag="sq")
                rawf = raw.rearrange("p h d -> p (h d)")
                nc.gpsimd.tensor_tensor(out=sq, in0=rawf, in1=rawf,
                                        op=Alu.mult)
                s2 = stp.tile([128, HPH], F32, tag="s2")
                nc.vector.tensor_reduce(
                    s2, sq.rearrange("p (h d) -> p h d", h=HPH), axis=X,
                    op=Alu.add)
                mu = stp.tile([128, HPH], F32, tag="mu")
                nc.vector.tensor_scalar(mu, sums, 1.0 / D, None, op0=Alu.mult)
                var = stp.tile([128, HPH], F32, tag="var")
                # var = s2/D - mu^2  (computed as (s2*(1/D) - mu*mu))
                nc.vector.tensor_tensor(out=var, in0=mu, in1=mu, op=Alu.mult)
                nc.vector.tensor_scalar(s2, s2, 1.0 / D, None, op0=Alu.mult)
                nc.vector.tensor_tensor(out=var, in0=s2, in1=var,
                                        op=Alu.subtract)
                sd = stp.tile([128, HPH], F32, tag="sd")
                nc.scalar.activation(sd, var, Act.Sqrt, bias=nc._epst[:, 0:1])
                rstd = stp.tile([128, HPH], F32, tag="rstd")
                nc.vector.reciprocal(rstd, sd)
                # ---- LN apply (gpsimd) ----
                ln = prep.tile([128, HPH, D], F32R, tag="ln")
                for h in range(HPH):
                    nc.gpsimd.tensor_scalar(
                        ln[:, h, :], raw[:, h, :], mu[:, h:h + 1],
                        rstd[:, h:h + 1], op0=Alu.subtract, op1=Alu.mult)
                # ---- RoPE: out = ln*COS + swap(ln)*SIN ----
                ctab = rtab[:, 2 * oc, t, :]      # cos_q or cos_k
                stab = rtab[:, 2 * oc + 1, t, :]  # sin_q or sin_k
                ra = prep.tile([128, HPH, D], F32R, tag="ra")
                nc.vector.tensor_tensor(out=ra, in0=ln,
                                        in1=_bcast_mid(ctab, HPH),
                                        op=Alu.mult)
                rb = prep.tile([128, HPH, D], F32R, tag="rb")
                half = D // 2
                nc.vector.tensor_tensor(
                    out=rb[:, :, 0:half], in0=ln[:, :, half:D],
                    in1=_bcast_mid(stab[:, 0:half], HPH), op=Alu.mult)
                nc.vector.tensor_tensor(
                    out=rb[:, :, half:D], in0=ln[:, :, 0:half],
                    in1=_bcast_mid(stab[:, half:D], HPH), op=Alu.mult)
                rot = prep.tile([128, HPH, D], F32R, tag="rot")
                nc.gpsimd.tensor_tensor(out=rot, in0=ra, in1=rb, op=Alu.add)
                # ---- transpose head pairs into QT/KT ----
                dst = QT if oc == 0 else KT
                for p in range(PAIRS):
                    tp = trps.tile([128, 128], F32R, tag="tr", name="qktr")
                    nc.tensor.transpose(
                        tp, rot.rearrange("p h d -> p (h d)")[:, p * 128:(p + 1) * 128],
                        ident[:])
                    if p % 2 == 0:
                        nc.scalar.copy(out=dst[:, p, t * 128:(t + 1) * 128],
                                       in_=tp)
                    else:
                        nc.vector.tensor_copy(
                            out=dst[:, p, t * 128:(t + 1) * 128], in_=tp)


def _attn_half(nc, tc, b, hh, QT, KT, V, ident, attn_t):
    """Attention for 8 heads of one half (processed as 4 head pairs).

    The two heads of a pair occupy partition ranges 0:64 / 64:128 of the
    QT/KT pair tiles, so their S^T matmuls land on distinct PE row-groups
    (tile_position auto (0,0)/(64,0)) and run concurrently when emitted
    back-to-back."""
    with (
        tc.tile_pool(name="pt", bufs=4) as ptp,
        tc.tile_pool(name="att", bufs=3) as attp,
    ):
        sps, s8ps, pvps, trp2 = nc._sps, nc._sps, nc._pvps, nc._trp
        identF = ident[0:D + 1, 0:D + 1].bitcast(F32)
        for pp in range(PAIRS):
            heads = (2 * pp, 2 * pp + 1)
            for qc in range(2):
                q0 = qc * 512
                pvs = [pvps.tile([D + 1, 512], F32, tag=f"pv{s}",
                                 name=f"pv{s}") for s in range(2)]
                for kt in range(8):
                    sp_pair = []
                    for s in range(2):
                        r = 64 * s
                        sp = sps.tile([128, 512], F32, tag="mm512",
                                      name="sp")
                        nc.tensor.matmul(
                            sp, KT[r:r + 64, pp, kt * 128:(kt + 1) * 128],
                            QT[r:r + 64, pp, q0:q0 + 512])
                        sp_pair.append(sp)
                    for s, hl in enumerate(heads):
                        pt = ptp.tile([128, 512], F32R, tag="pt")
                        nc.scalar.activation(pt, sp_pair[s][:], Act.Exp)
                        nc.tensor.matmul(pvs[s], V[:, kt, hl, :], pt,
                                         start=(kt == 0), stop=False)
                for s, hl in enumerate(heads):
                    r = 64 * s
                    s8 = s8ps.tile([1, 512], F32, tag="mm512", name="s8")
                    nc.tensor.matmul(s8, KT[r:r + 64, pp, 1024:1025],
                                     QT[r:r + 64, pp, q0:q0 + 512])
                    pt8 = ptp.tile([1, 512], F32R, tag="pt8")
                    nc.scalar.activation(pt8, s8[:], Act.Exp)
                    nc.tensor.matmul(pvs[s], V[0:1, 8, hl, :], pt8,
                                     start=False, stop=True)
                for s, hl in enumerate(heads):
                    hg = hh * HPH + hl
                    pvsb = attp.tile([D + 1, 512], F32R, tag="pvs")
                    nc.vector.tensor_copy(out=pvsb, in_=pvs[s])
                    for j in range(4):
                        trf = trp2.tile([128, 128], F32R, tag="tr",
                                        name="atr")
                        tr = trf[:, 0:D + 1].bitcast(F32)
                        nc.tensor.transpose(
                            tr, pvsb[:, j * 128:(j + 1) * 128].bitcast(F32),
                            identF)
                        rl = attp.tile([128, 1], F32, tag="rl")
                        nc.vector.reciprocal(rl, tr[:, D:D + 1])
                        stage = attp.tile([128, D], F32R, tag="stage")
                        nc.vector.tensor_scalar(stage, tr[:, 0:D],
                                                rl[:, 0:1], None,
                                                op0=Alu.mult)
                        tok = q0 + j * 128
                        nc.sync.dma_start(
                            out=attn_t[tok:tok + 128, hg * D:(hg + 1) * D],
                            in_=stage)
            # ---- stragglers: q tokens 1023:1025 (token 1023 redone) ----
            for s, hl in enumerate(heads):
                r = 64 * s
                hg = hh * HPH + hl
                qstr = QT[r:r + 64, pp, 1023:1025]
                kstr = KT[r:r + 64, pp, 1024:1025]
                sp1 = s8ps.tile([128, 18], F32, tag="mm512", name="sp1")
                for kt in range(8):
                    nc.tensor.matmul(
                        sp1[:, 2 * kt:2 * kt + 2],
                        KT[r:r + 64, pp, kt * 128:(kt + 1) * 128], qstr)
                nc.tensor.matmul(sp1[0:1, 16:18], kstr, qstr)
                p1 = ptp.tile([128, 18], F32R, tag="p1")
                nc.scalar.activation(p1, sp1[:], Act.Exp)
                pv1 = pvps.tile([D + 1, 2], F32, tag=f"pv{s}",
                                name=f"pv1_{s}")
                for kt in range(8):
                    nc.tensor.matmul(pv1, V[:, kt, hl, :],
                                     p1[:, 2 * kt:2 * kt + 2],
                                     start=(kt == 0), stop=False)
                nc.tensor.matmul(pv1, V[0:1, 8, hl, :], p1[0:1, 16:18],
                                 start=False, stop=True)
                pvs1 = attp.tile([D + 1, 2], F32R, tag="pvs")
                nc.vector.tensor_copy(out=pvs1, in_=pv1)
                trf1 = trp2.tile([128, 128], F32R, tag="tr", name="atr1")
                tr1 = trf1[:, 0:D + 1].bitcast(F32)
                nc.tensor.transpose(tr1[0:2, :], pvs1.bitcast(F32), identF)
                rl1 = attp.tile([128, 1], F32, tag="rl")
                nc.vector.reciprocal(rl1[0:2, :], tr1[0:2, D:D + 1])
                stage1 = attp.tile([128, D], F32R, tag="stage")
                nc.vector.tensor_scalar(stage1[0:2, :], tr1[0:2, 0:D],
                                        rl1[0:2, 0:1], None, op0=Alu.mult)
                nc.sync.dma_start(
                    out=attn_t[1023:1025, hg * D:(hg + 1) * D],
                    in_=stage1[0:2, :])


def _norm_proj(nc, tc, b, pwt, attn_t, ident, y, pbb):
    """scale_norm over C + proj matmul + output DMA for batch b."""
    with (
        tc.tile_pool(name="lnt", bufs=1) as lntp,
        tc.tile_pool(name="ain", bufs=2) as ainp,
        tc.tile_pool(name="lst", bufs=4) as lstp,
    ):
        lps = nc._mmps
        lnT = lntp.tile([128, 8, NPAD], F32R)
        for t in range(NT):
            rows = 128 if t < NT - 1 else N - 128 * (NT - 1)
            a = ainp.tile([128, C], F32R, tag="a")
            nc.sync.dma_start(out=a[:rows, :],
                              in_=attn_t[t * 128:t * 128 + rows, :])
            s = lstp.tile([128, 1], F32, tag="s")
            nc.vector.tensor_reduce(s, a, axis=X, op=Alu.add)
            sq = ainp.tile([128, C], F32, tag="lsq")
            nc.gpsimd.tensor_tensor(out=sq, in0=a, in1=a, op=Alu.mult)
            s2 = lstp.tile([128, 1], F32, tag="ls2")
            nc.vector.tensor_reduce(s2, sq, axis=X, op=Alu.add)
            mu = lstp.tile([128, 1], F32, tag="lmu")
            nc.vector.tensor_scalar(mu, s, 1.0 / C, None, op0=Alu.mult)
            var = lstp.tile([128, 1], F32, tag="lvar")
            nc.vector.tensor_tensor(out=var, in0=mu, in1=mu, op=Alu.mult)
            nc.vector.tensor_scalar(s2, s2, 1.0 / C, None, op0=Alu.mult)
            nc.vector.tensor_tensor(out=var, in0=s2, in1=var,
                                    op=Alu.subtract)
            sd = lstp.tile([128, 1], F32, tag="lsd")
            nc.scalar.activation(sd, var, Act.Sqrt, bias=nc._epst[:, 0:1])
            rstd = lstp.tile([128, 1], F32, tag="lrstd")
            nc.vector.reciprocal(rstd, sd)
            ln = ainp.tile([128, C], F32R, tag="ln2")
            nc.vector.tensor_scalar(ln, a, mu[:, 0:1], rstd[:, 0:1],
                                    op0=Alu.subtract, op1=Alu.mult)
            if True:
                ltps = nc._trp
                for k in range(8):
                    tp = ltps.tile([128, 128], F32R, tag="tr", name="lntr")
                    nc.tensor.transpose(tp, ln[:, k * 128:(k + 1) * 128],
                                        ident[:])
                    if k % 2 == 0:
                        nc.scalar.copy(out=lnT[:, k, t * 128:(t + 1) * 128],
                                       in_=tp)
                    else:
                        nc.vector.tensor_copy(
                            out=lnT[:, k, t * 128:(t + 1) * 128], in_=tp)
        with tc.tile_pool(name="pwp", bufs=2) as pwp:
            for oc in range(2):
                wchunk = pwp.tile([128, 8, 512], F32R)
                nc.sync.dma_start(
                    out=wchunk,
                    in_=pwt[:, oc * 512:(oc + 1) * 512].rearrange(
                        "(k p) o -> p k o", p=128))
                for t in range(NT):
                    ps = lps.tile([128, 512], F32, tag="mm512",
                                  name="projps")
                    for k in range(8):
                        nc.tensor.matmul(ps, lnT[:, k, t * 128:(t + 1) * 128],
                                         wchunk[:, k, :], start=(k == 0),
                                         stop=(k == 7))
                    ostage = ainp.tile([128, 512], F32, tag="ostage")
                    if pbb is not None:
                        nc.vector.tensor_tensor(
                            out=ostage, in0=ps,
                            in1=pbb[:, oc * 512:(oc + 1) * 512], op=Alu.add)
                    else:
                        nc.scalar.copy(out=ostage, in_=ps)
                    rows = 128 if t < NT - 1 else N - 128 * (NT - 1)
                    nc.sync.dma_start(
                        out=y[b, t * 128:t * 128 + rows,
                              oc * 512:(oc + 1) * 512],
                        in_=ostage[:rows, :])


def _host_prep(inputs):
    """Precompute permuted/transposed weights and folded rope tables."""
    perm = np.concatenate([np.arange(0, D, 2), np.arange(1, D, 2)])
    swap = np.concatenate([np.arange(D // 2, D), np.arange(0, D // 2)])

    qkv_w = np.asarray(inputs["qkv_w"], np.float32)
    rope = np.asarray(inputs["rope"], np.float32)
    sin_t, cos_t = rope[:, :D], rope[:, D:]

    # column order: [half][q|k|v][head-in-half][d]  (d permuted for q,k)
    row_order = np.empty(3 * C, np.int64)
    col = 0
    for hh in range(HH):
        for grp in range(3):
            for h in range(hh * HPH, (hh + 1) * HPH):
                base = grp * C + h * D
                idx = base + (perm if grp < 2 else np.arange(D))
                row_order[col:col + D] = idx
                col += D
    wt = np.ascontiguousarray(qkv_w[row_order, :].T)  # [C, 3C]

    qb = np.asarray(inputs["q_bias"], np.float32)
    kb = np.asarray(inputs["k_bias"], np.float32)
    vb = np.asarray(inputs["v_bias"], np.float32)
    full_bias = np.concatenate([qb, kb, vb])
    qkvb = full_bias[row_order].astype(np.float32)

    def make_tables(g, scale):
        gp = np.asarray(g, np.float32)[perm]          # g in permuted coords
        cos_p = cos_t[:, perm]                        # [1024, D]
        sin_p = sin_t[:, perm]
        sgn = np.where(np.arange(D) < D // 2, -1.0, 1.0).astype(np.float32)
        cost = np.zeros((NPAD, D), np.float32)
        sint = np.zeros((NPAD, D), np.float32)
        cost[0] = gp * scale
        cost[1:N] = cos_p * gp[None, :] * scale
        sint[1:N] = sin_p * sgn[None, :] * gp[swap][None, :] * scale
        return cost, sint

    cq, sq_ = make_tables(inputs["qn_g"], SCALE)
    ck, sk = make_tables(inputs["kn_g"], 1.0)
    ropet = np.stack([cq, sq_, ck, sk])  # [4, NPAD, D]

    norm_g = np.asarray(inputs["norm_g"], np.float32)
    norm_b = np.asarray(inputs["norm_b"], np.float32)
    proj_w = np.asarray(inputs["proj_w"], np.float32)
    proj_b = np.asarray(inputs["proj_b"], np.float32)
    pwt = np.ascontiguousarray((proj_w * norm_g[None, :]).T)  # [C, C]
    pbias = (proj_b + norm_b @ proj_w.T).astype(np.float32)

    return wt, qkvb, ropet, pwt, pbias


def kernel(**inputs):
    qn_b = np.asarray(inputs["qn_b"], np.float32)
    kn_b = np.asarray(inputs["kn_b"], np.float32)
    assert not qn_b.any() and not kn_b.any(), \
        "kernel specialized for qn_b == kn_b == 0"

    wt, qkvb, ropet, pwt, pbias = _host_prep(inputs)
    has_kbias = bool(np.asarray(inputs["k_bias"]).any())
    has_pbias = bool(pbias.any())

    key = (has_kbias, has_pbias)
    if key not in _CACHE:
        _CACHE[key] = _build(has_kbias, has_pbias)
    nc = _CACHE[key]

    x = np.asarray(inputs["x"], np.float32)
    in_maps = []
    for c in range(NCORES):
        in_maps.append({
            "x": np.ascontiguousarray(x[c * BL:(c + 1) * BL]),
            "wt": wt, "qkvb": qkvb, "ropet": ropet, "pwt": pwt,
            "pbias": pbias, "ident": np.eye(128, dtype=np.float32),
            "onesd": np.ones(1, dtype=np.float32),
        })
    res = run_bass_kernel_spmd(nc, in_maps, core_ids=list(range(NCORES)))
    out = np.concatenate([res.results[c]["y"] for c in range(NCORES)], axis=0)
    return out.astype(np.float32)


# revision 24
# speedup vs baseline: 973.8306x; 586.2541x over previous
"""EvaAttention TRN2 kernel: data-parallel over batch across 8 NeuronCores.

Per core (2 batches): qkv proj (fp32r matmuls), per-head QK layernorm + RoPE
(folded into host-precomputed cos/sin tables incl. scale and qn_g), attention
with no-max-subtraction softmax computed entirely in S^T layout (softmax
denominator via ones-augmented V column), scale_norm + proj.
"""
import os
import sys

for _p in (
    "/root/.axon_site",
    "/root/.axon_site/_ro/trn_rl_repo",
    "/root/.axon_site/_ro/pypackages",
    "/opt/trn_rl_repo",
    "/opt/pypackages",
):
    if os.path.isdir(_p) and _p not in sys.path:
        sys.path.append(_p)

import numpy as np

import concourse.bass as bass
import concourse.bacc as bacc
import concourse.tile as tile
from concourse import mybir, masks
from concourse.bass_utils import run_bass_kernel_spmd

F32 = mybir.dt.float32
F32R = mybir.dt.float32r
Act = mybir.ActivationFunctionType
Alu = mybir.AluOpType
X = mybir.AxisListType.X

B, N, C, H, D = 16, 1025, 1024, 16, 64
EPS = 1e-6
SCALE = D ** -0.5
NCORES = 8
BL = B // NCORES          # batches per core
NT = 9                    # token tiles per batch (pad 1025 -> 1152)
NPAD = NT * 128
HH = 2                    # head halves
HPH = H // HH             # heads per half (8)
PAIRS = HPH // 2          # head pairs per half (4)

_CACHE = {}


def _bcast_mid(ap2d, n):
    """[P, F] AP -> [P, n, F] with step-0 middle dim (free-dim broadcast)."""
    return bass.AP(tensor=ap2d.tensor, offset=ap2d.offset,
                   ap=[ap2d.ap[0], [0, n], ap2d.ap[1]])


def _build(has_kbias, has_pbias, repeat=1):
    nc = bacc.Bacc("TRN2", target_bir_lowering=False, debug=False,
                   num_devices=NCORES)

    x_in = nc.dram_tensor("x", [BL, N, C], F32R, kind="ExternalInput").ap()
    wt = nc.dram_tensor("wt", [C, 3 * C], F32R, kind="ExternalInput").ap()
    qkvb = nc.dram_tensor("qkvb", [3 * C], F32R, kind="ExternalInput").ap()
    ropet = nc.dram_tensor("ropet", [4, NPAD, D], F32R, kind="ExternalInput").ap()
    pwt = nc.dram_tensor("pwt", [C, C], F32R, kind="ExternalInput").ap()
    pbias = nc.dram_tensor("pbias", [C], F32R, kind="ExternalInput").ap()
    ident_d = nc.dram_tensor("ident", [128, 128], F32R,
                             kind="ExternalInput").ap()
    onesd = nc.dram_tensor("onesd", [1], F32R, kind="ExternalInput").ap()
    y = nc.dram_tensor("y", [BL, N, C], F32, kind="ExternalOutput").ap()

    with tile.TileContext(nc, pool_alloc_mode="queue") as tc:
        with tc.tile_pool(name="consts", bufs=1) as consts:
            ident = consts.tile([128, 128], F32R)
            nc.sync.dma_start(out=ident, in_=ident_d)
            epst = consts.tile([128, 1], F32)
            nc.vector.memset(epst, EPS)
            # rope tables: [0]=cos_q [1]=sin_q [2]=cos_k [3]=sin_k
            rtab = consts.tile([128, 4, NT, D], F32R)
            nc.sync.dma_start(
                out=rtab, in_=ropet.rearrange("f (t p) d -> p f t d", p=128))
            biasb = consts.tile([128, 3 * C], F32R)
            nc.sync.dma_start(
                out=biasb,
                in_=bass.AP(tensor=qkvb.tensor, offset=qkvb.offset,
                            ap=[[0, 128], [1, 3 * C]]))
            if has_pbias:
                pbb = consts.tile([128, C], F32R)
                nc.sync.dma_start(
                    out=pbb,
                    in_=bass.AP(tensor=pbias.tensor, offset=pbias.offset,
                                ap=[[0, 128], [1, C]]))

            nc._epst = epst
            nc._onesd = onesd
            import contextlib
            _psctx = contextlib.ExitStack()
            nc._mmps = _psctx.enter_context(
                tc.tile_pool(name="mmps", bufs=4, space="PSUM"))
            nc._sps = nc._mmps
            nc._pvps = _psctx.enter_context(
                tc.tile_pool(name="pvpsg", bufs=1, space="PSUM"))
            nc._trp = _psctx.enter_context(
                tc.tile_pool(name="trpg", bufs=2, space="PSUM"))
            for _rep in range(repeat):
                for b in range(BL):
                    _batch(nc, tc, b, x_in, wt, pwt, y, ident, rtab,
                           biasb, pbb if has_pbias else None, has_kbias)
            _psctx.close()
    nc.compile()
    return nc


def _batch(nc, tc, b, x_in, wt, pwt, y, ident, rtab, biasb, pbb,
           has_kbias):
    with tc.tile_pool(name="adram", bufs=1, space="DRAM") as adp:
        attn_t = adp.tile([NPAD, C], F32)
        with tc.tile_pool(name="xt", bufs=1) as xtp:
            xT = [xtp.tile([128, NPAD], F32R, tag=f"xt{k}", name=f"xt{k}")
                  for k in range(8)]
            _build_xt(nc, tc, b, x_in, ident, xT)
            for hh in range(HH):
                with tc.tile_pool(name="qkt", bufs=1) as qktp:
                    QT = qktp.tile([128, PAIRS, NPAD], F32R)
                    KT = qktp.tile([128, PAIRS, NPAD], F32R)
                    V = qktp.tile([128, NT, HPH, D + 1], F32R)
                    _qkv_half(nc, tc, b, hh, wt, xT, ident, rtab, biasb, V,
                              QT, KT, has_kbias)
                    _attn_half(nc, tc, b, hh, QT, KT, V, ident, attn_t)
        _norm_proj(nc, tc, b, pwt, attn_t, ident, y, pbb)


def _build_xt(nc, tc, b, x_in, ident, xT):
    """Load x[b] and PE-transpose into xT [128c, 8k, NPAD tok]."""
    with tc.tile_pool(name="xraw", bufs=3) as xrp:
        xps = nc._trp
        for t in range(NT):
            xraw = xrp.tile([128, C], F32R)
            rows = 128 if t < NT - 1 else N - 128 * (NT - 1)
            nc.sync.dma_start(out=xraw[:rows, :],
                              in_=x_in[b, t * 128:t * 128 + rows, :])
            for k in range(8):
                ps = xps.tile([128, 128], F32R, tag="tr", name="xtr")
                nc.tensor.transpose(ps, xraw[:, k * 128:(k + 1) * 128],
                                    ident[:])
                if k % 2 == 0:
                    nc.scalar.copy(out=xT[k][:, t * 128:(t + 1) * 128],
                                   in_=ps)
                else:
                    nc.vector.tensor_copy(
                        out=xT[k][:, t * 128:(t + 1) * 128], in_=ps)


def _qkv_half(nc, tc, b, hh, wt, xT, ident, rtab, biasb, V, QT, KT,
              has_kbias):
    """qkv matmuls for one head-half + LN + RoPE + transposes into QT/KT/V."""
    with (
        tc.tile_pool(name="wp", bufs=2) as wp,
        tc.tile_pool(name="prep", bufs=2) as prep,
        tc.tile_pool(name="stat", bufs=4) as stp,
    ):
        qps, trps = nc._mmps, nc._trp
        # ones column of V (col D); untouched pad rows are never read
        nc.sync.dma_start(
            out=V[:, :, :, D:D + 1].rearrange("p t h o -> p (t h) o"),
            in_=bass.AP(tensor=nc._onesd.tensor, offset=nc._onesd.offset,
                        ap=[[0, 128], [0, NT * HPH], [1, 1]]))
        for oc in range(3):  # 0=q cols, 1=k cols, 2=v cols
            col0 = hh * 1536 + oc * 512
            wchunk = wp.tile([128, 8, 512], F32R)
            nc.sync.dma_start(
                out=wchunk,
                in_=wt[:, col0:col0 + 512].rearrange("(k p) o -> p k o",
                                                     p=128))
            for t in range(NT):
                ps = qps.tile([128, 512], F32, tag="mm512", name="qkvps")
                for k in range(8):
                    nc.tensor.matmul(ps, xT[k][:, t * 128:(t + 1) * 128],
                                     wchunk[:, k, :], start=(k == 0),
                                     stop=(k == 7))
                if oc == 2:
                    # v: bias add straight into V tile
                    nc.vector.tensor_tensor(
                        out=V[:, t, :, 0:D],
                        in0=ps.rearrange("p (h d) -> p h d", h=HPH),
                        in1=biasb[:, col0:col0 + 512].rearrange(
                            "p (h d) -> p h d", h=HPH),
                        op=Alu.add)
                    continue
                raw = prep.tile([128, HPH, D], F32, tag="raw")
                if oc == 1 and not has_kbias:
                    nc.scalar.copy(out=raw.rearrange("p h d -> p (h d)"),
                                   in_=ps)
                else:
                    nc.vector.tensor_tensor(
                        out=raw.rearrange("p h d -> p (h d)"), in0=ps,
                        in1=biasb[:, col0:col0 + 512], op=Alu.add)
                # ---- stats: mu, rstd per head ----
                sums = stp.tile([128, HPH], F32, tag="sums")
                nc.vector.tensor_reduce(sums, raw, axis=X, op=Alu.add)
                sq = prep.tile([128, HPH * D], F32, tag="sq")
                rawf = raw.rearrange("p h d -> p (h d)")
                nc.gpsimd.tensor_tensor(out=sq, in0=rawf, in1=rawf,
                                        op=Alu.mult)
                s2 = stp.tile([128, HPH], F32, tag="s2")
                nc.vector.tensor_reduce(
                    s2, sq.rearrange("p (h d) -> p h d", h=HPH), axis=X,
                    op=Alu.add)
                mu = stp.tile([128, HPH], F32, tag="mu")
                nc.vector.tensor_scalar(mu, sums, 1.0 / D, None, op0=Alu.mult)
                var = stp.tile([128, HPH], F32, tag="var")
                # var = s2/D - mu^2  (computed as (s2*(1/D) - mu*mu))
                nc.vector.tensor_tensor(out=var, in0=mu, in1=mu, op=Alu.mult)
                nc.vector.tensor_scalar(s2, s2, 1.0 / D, None, op0=Alu.mult)
                nc.vector.tensor_tensor(out=var, in0=s2, in1=var,
                                        op=Alu.subtract)
                sd = stp.tile([128, HPH], F32, tag="sd")
                nc.scalar.activation(sd, var, Act.Sqrt, bias=nc._epst[:, 0:1])
                rstd = stp.tile([128, HPH], F32, tag="rstd")
                nc.vector.reciprocal(rstd, sd)
                # ---- LN apply (gpsimd) ----
                ln = prep.tile([128, HPH, D], F32, tag="ln")
                for h in range(HPH):
                    nc.gpsimd.tensor_scalar(
                        ln[:, h, :], raw[:, h, :], mu[:, h:h + 1],
                        rstd[:, h:h + 1], op0=Alu.subtract, op1=Alu.mult)
                # ---- RoPE: out = ln*COS + swap(ln)*SIN ----
                ctab = rtab[:, 2 * oc, t, :]      # cos_q or cos_k
                stab = rtab[:, 2 * oc + 1, t, :]  # sin_q or sin_k
                ra = prep.tile([128, HPH, D], F32, tag="ra")
                nc.vector.tensor_tensor(out=ra, in0=ln,
                                        in1=_bcast_mid(ctab, HPH),
                                        op=Alu.mult)
                rb = prep.tile([128, HPH, D], F32, tag="rb")
                half = D // 2
                nc.vector.tensor_tensor(
                    out=rb[:, :, 0:half], in0=ln[:, :, half:D],
                    in1=_bcast_mid(stab[:, 0:half], HPH), op=Alu.mult)
                nc.vector.tensor_tensor(
                    out=rb[:, :, half:D], in0=ln[:, :, 0:half],
                    in1=_bcast_mid(stab[:, half:D], HPH), op=Alu.mult)
                rot = prep.tile([128, HPH, D], F32R, tag="rot")
                nc.gpsimd.tensor_tensor(out=rot, in0=ra, in1=rb, op=Alu.add)
                # ---- transpose head pairs into QT/KT ----
                dst = QT if oc == 0 else KT
                for p in range(PAIRS):
                    tp = trps.tile([128, 128], F32R, tag="tr", name="qktr")
                    nc.tensor.transpose(
                        tp, rot.rearrange("p h d -> p (h d)")[:, p * 128:(p + 1) * 128],
                        ident[:])
                    if p % 2 == 0:
                        nc.scalar.copy(out=dst[:, p, t * 128:(t + 1) * 128],
                                       in_=tp)
                    else:
                        nc.vector.tensor_copy(
                            out=dst[:, p, t * 128:(t + 1) * 128], in_=tp)


def _attn_half(nc, tc, b, hh, QT, KT, V, ident, attn_t):
    """Attention for 8 heads of one half (processed as 4 head pairs).

    The two heads of a pair occupy partition ranges 0:64 / 64:128 of the
    QT/KT pair tiles, so their S^T matmuls land on distinct PE row-groups
    (tile_position auto (0,0)/(64,0)) and run concurrently when emitted
    back-to-back."""
    with (
        tc.tile_pool(name="pt", bufs=4) as ptp,
        tc.tile_pool(name="att", bufs=3) as attp,
    ):
        sps, s8ps, pvps, trp2 = nc._sps, nc._sps, nc._pvps, nc._trp
        identF = ident[0:D + 1, 0:D + 1].bitcast(F32)
        for pp in range(PAIRS):
            heads = (2 * pp, 2 * pp + 1)
            for qc in range(2):
                q0 = qc * 512
                pvs = [pvps.tile([D + 1, 512], F32, tag=f"pv{s}",
                                 name=f"pv{s}") for s in range(2)]
                for kt in range(8):
                    sp_pair = []
                    for s in range(2):
                        r = 64 * s
                        sp = sps.tile([128, 512], F32, tag="mm512",
                                      name="sp")
                        nc.tensor.matmul(
                            sp, KT[r:r + 64, pp, kt * 128:(kt + 1) * 128],
                            QT[r:r + 64, pp, q0:q0 + 512])
                        sp_pair.append(sp)
                    for s, hl in enumerate(heads):
                        pt = ptp.tile([128, 512], F32R, tag="pt")
                        nc.scalar.activation(pt, sp_pair[s][:], Act.Exp)
                        nc.tensor.matmul(pvs[s], V[:, kt, hl, :], pt,
                                         start=(kt == 0), stop=False)
                for s, hl in enumerate(heads):
                    r = 64 * s
                    s8 = s8ps.tile([1, 512], F32, tag="mm512", name="s8")
                    nc.tensor.matmul(s8, KT[r:r + 64, pp, 1024:1025],
                                     QT[r:r + 64, pp, q0:q0 + 512])
                    pt8 = ptp.tile([1, 512], F32R, tag="pt8")
                    nc.scalar.activation(pt8, s8[:], Act.Exp)
                    nc.tensor.matmul(pvs[s], V[0:1, 8, hl, :], pt8,
                                     start=False, stop=True)
                for s, hl in enumerate(heads):
                    hg = hh * HPH + hl
                    pvsb = attp.tile([D + 1, 512], F32, tag="pvs")
                    nc.vector.tensor_copy(out=pvsb, in_=pvs[s])
                    for j in range(4):
                        trf = trp2.tile([128, 128], F32R, tag="tr",
                                        name="atr")
                        tr = trf[:, 0:D + 1].bitcast(F32)
                        nc.tensor.transpose(
                            tr, pvsb[:, j * 128:(j + 1) * 128],
                            identF)
                        rl = attp.tile([128, 1], F32, tag="rl")
                        nc.vector.reciprocal(rl, tr[:, D:D + 1])
                        stage = attp.tile([128, D], F32, tag="stage")
                        nc.vector.tensor_scalar(stage, tr[:, 0:D],
                                                rl[:, 0:1], None,
                                                op0=Alu.mult)
                        tok = q0 + j * 128
                        nc.sync.dma_start(
                            out=attn_t[tok:tok + 128, hg * D:(hg + 1) * D],
                            in_=stage)
            # ---- stragglers: q tokens 1023:1025 (token 1023 redone) ----
            for s, hl in enumerate(heads):
                r = 64 * s
                hg = hh * HPH + hl
                qstr = QT[r:r + 64, pp, 1023:1025]
                kstr = KT[r:r + 64, pp, 1024:1025]
                sp1 = s8ps.tile([128, 18], F32, tag="mm512", name="sp1")
                for kt in range(8):
                    nc.tensor.matmul(
                        sp1[:, 2 * kt:2 * kt + 2],
                        KT[r:r + 64, pp, kt * 128:(kt + 1) * 128], qstr)
                nc.tensor.matmul(sp1[0:1, 16:18], kstr, qstr)
                p1 = ptp.tile([128, 18], F32R, tag="p1")
                nc.scalar.activation(p1, sp1[:], Act.Exp)
                pv1 = pvps.tile([D + 1, 2], F32, tag=f"pv{s}",
                                name=f"pv1_{s}")
                for kt in range(8):
                    nc.tensor.matmul(pv1, V[:, kt, hl, :],
                                     p1[:, 2 * kt:2 * kt + 2],
                                     start=(kt == 0), stop=False)
                nc.tensor.matmul(pv1, V[0:1, 8, hl, :], p1[0:1, 16:18],
                                 start=False, stop=True)
                pvs1 = attp.tile([D + 1, 2], F32, tag="pvs")
                nc.vector.tensor_copy(out=pvs1, in_=pv1)
                trf1 = trp2.tile([128, 128], F32R, tag="tr", name="atr1")
                tr1 = trf1[:, 0:D + 1].bitcast(F32)
                nc.tensor.transpose(tr1[0:2, :], pvs1, identF)
                rl1 = attp.tile([128, 1], F32, tag="rl")
                nc.vector.reciprocal(rl1[0:2, :], tr1[0:2, D:D + 1])
                stage1 = attp.tile([128, D], F32, tag="stage")
                nc.vector.tensor_scalar(stage1[0:2, :], tr1[0:2, 0:D],
                                        rl1[0:2, 0:1], None, op0=Alu.mult)
                nc.sync.dma_start(
                    out=attn_t[1023:1025, hg * D:(hg + 1) * D],
                    in_=stage1[0:2, :])


def _norm_proj(nc, tc, b, pwt, attn_t, ident, y, pbb):
    """scale_norm over C + proj matmul + output DMA for batch b."""
    with (
        tc.tile_pool(name="lnt", bufs=1) as lntp,
        tc.tile_pool(name="ain", bufs=2) as ainp,
        tc.tile_pool(name="lst", bufs=4) as lstp,
    ):
        lps = nc._mmps
        lnT = lntp.tile([128, 8, NPAD], F32R)
        for t in range(NT):
            rows = 128 if t < NT - 1 else N - 128 * (NT - 1)
            a = ainp.tile([128, C], F32, tag="a")
            nc.sync.dma_start(out=a[:rows, :],
                              in_=attn_t[t * 128:t * 128 + rows, :])
            s = lstp.tile([128, 1], F32, tag="s")
            nc.vector.tensor_reduce(s, a, axis=X, op=Alu.add)
            sq = ainp.tile([128, C], F32, tag="lsq")
            nc.gpsimd.tensor_tensor(out=sq, in0=a, in1=a, op=Alu.mult)
            s2 = lstp.tile([128, 1], F32, tag="ls2")
            nc.vector.tensor_reduce(s2, sq, axis=X, op=Alu.add)
            mu = lstp.tile([128, 1], F32, tag="lmu")
            nc.vector.tensor_scalar(mu, s, 1.0 / C, None, op0=Alu.mult)
            var = lstp.tile([128, 1], F32, tag="lvar")
            nc.vector.tensor_tensor(out=var, in0=mu, in1=mu, op=Alu.mult)
            nc.vector.tensor_scalar(s2, s2, 1.0 / C, None, op0=Alu.mult)
            nc.vector.tensor_tensor(out=var, in0=s2, in1=var,
                                    op=Alu.subtract)
            sd = lstp.tile([128, 1], F32, tag="lsd")
            nc.scalar.activation(sd, var, Act.Sqrt, bias=nc._epst[:, 0:1])
            rstd = lstp.tile([128, 1], F32, tag="lrstd")
            nc.vector.reciprocal(rstd, sd)
            ln = ainp.tile([128, C], F32R, tag="ln2")
            nc.vector.tensor_scalar(ln, a, mu[:, 0:1], rstd[:, 0:1],
                                    op0=Alu.subtract, op1=Alu.mult)
            if True:
                ltps = nc._trp
                for k in range(8):
                    tp = ltps.tile([128, 128], F32R, tag="tr", name="lntr")
                    nc.tensor.transpose(tp, ln[:, k * 128:(k + 1) * 128],
                                        ident[:])
                    if k % 2 == 0:
                        nc.scalar.copy(out=lnT[:, k, t * 128:(t + 1) * 128],
                                       in_=tp)
                    else:
                        nc.vector.tensor_copy(
                            out=lnT[:, k, t * 128:(t + 1) * 128], in_=tp)
        with tc.tile_pool(name="pwp", bufs=2) as pwp:
            for oc in range(2):
                wchunk = pwp.tile([128, 8, 512], F32R)
                nc.sync.dma_start(
                    out=wchunk,
                    in_=pwt[:, oc * 512:(oc + 1) * 512].rearrange(
                        "(k p) o -> p k o", p=128))
                for t in range(NT):
                    ps = lps.tile([128, 512], F32, tag="mm512",
                                  name="projps")
                    for k in range(8):
                        nc.tensor.matmul(ps, lnT[:, k, t * 128:(t + 1) * 128],
                                         wchunk[:, k, :], start=(k == 0),
                                         stop=(k == 7))
                    ostage = ainp.tile([128, 512], F32, tag="ostage")
                    if pbb is not None:
                        nc.vector.tensor_tensor(
                            out=ostage, in0=ps,
                            in1=pbb[:, oc * 512:(oc + 1) * 512], op=Alu.add)
                    else:
                        nc.scalar.copy(out=ostage, in_=ps)
                    rows = 128 if t < NT - 1 else N - 128 * (NT - 1)
                    nc.sync.dma_start(
                        out=y[b, t * 128:t * 128 + rows,
                              oc * 512:(oc + 1) * 512],
                        in_=ostage[:rows, :])


def _host_prep(inputs):
    """Precompute permuted/transposed weights and folded rope tables."""
    perm = np.concatenate([np.arange(0, D, 2), np.arange(1, D, 2)])
    swap = np.concatenate([np.arange(D // 2, D), np.arange(0, D // 2)])

    qkv_w = np.asarray(inputs["qkv_w"], np.float32)
    rope = np.asarray(inputs["rope"], np.float32)
    sin_t, cos_t = rope[:, :D], rope[:, D:]

    # column order: [half][q|k|v][head-in-half][d]  (d permuted for q,k)
    row_order = np.empty(3 * C, np.int64)
    col = 0
    for hh in range(HH):
        for grp in range(3):
            for h in range(hh * HPH, (hh + 1) * HPH):
                base = grp * C + h * D
                idx = base + (perm if grp < 2 else np.arange(D))
                row_order[col:col + D] = idx
                col += D
    wt = np.ascontiguousarray(qkv_w[row_order, :].T)  # [C, 3C]

    qb = np.asarray(inputs["q_bias"], np.float32)
    kb = np.asarray(inputs["k_bias"], np.float32)
    vb = np.asarray(inputs["v_bias"], np.float32)
    full_bias = np.concatenate([qb, kb, vb])
    qkvb = full_bias[row_order].astype(np.float32)

    def make_tables(g, scale):
        gp = np.asarray(g, np.float32)[perm]          # g in permuted coords
        cos_p = cos_t[:, perm]                        # [1024, D]
        sin_p = sin_t[:, perm]
        sgn = np.where(np.arange(D) < D // 2, -1.0, 1.0).astype(np.float32)
        cost = np.zeros((NPAD, D), np.float32)
        sint = np.zeros((NPAD, D), np.float32)
        cost[0] = gp * scale
        cost[1:N] = cos_p * gp[None, :] * scale
        sint[1:N] = sin_p * sgn[None, :] * gp[swap][None, :] * scale
        return cost, sint

    cq, sq_ = make_tables(inputs["qn_g"], SCALE)
    ck, sk = make_tables(inputs["kn_g"], 1.0)
    ropet = np.stack([cq, sq_, ck, sk])  # [4, NPAD, D]

    norm_g = np.asarray(inputs["norm_g"], np.float32)
    norm_b = np.asarray(inputs["norm_b"], np.float32)
    proj_w = np.asarray(inputs["proj_w"], np.float32)
    proj_b = np.asarray(inputs["proj_b"], np.float32)
    pwt = np.ascontiguousarray((proj_w * norm_g[None, :]).T)  # [C, C]
    pbias = (proj_b + norm_b @ proj_w.T).astype(np.float32)

    return wt, qkvb, ropet, pwt, pbias


def kernel(**inputs):
    qn_b = np.asarray(inputs["qn_b"], np.float32)
    kn_b = np.asarray(inputs["kn_b"], np.float32)
    assert not qn_b.any() and not kn_b.any(), \
        "kernel specialized for qn_b == kn_b == 0"

    wt, qkvb, ropet, pwt, pbias = _host_prep(inputs)
    has_kbias = bool(np.asarray(inputs["k_bias"]).any())
    has_pbias = bool(pbias.any())

    key = (has_kbias, has_pbias)
    if key not in _CACHE:
        _CACHE[key] = _build(has_kbias, has_pbias)
    nc = _CACHE[key]

    x = np.asarray(inputs["x"], np.float32)
    in_maps = []
    for c in range(NCORES):
        in_maps.append({
            "x": np.ascontiguousarray(x[c * BL:(c + 1) * BL]),
            "wt": wt, "qkvb": qkvb, "ropet": ropet, "pwt": pwt,
            "pbias": pbias, "ident": np.eye(128, dtype=np.float32),
            "onesd": np.ones(1, dtype=np.float32),
        })
    res = run_bass_kernel_spmd(nc, in_maps, core_ids=list(range(NCORES)))
    out = np.concatenate([res.results[c]["y"] for c in range(NCORES)], axis=0)
    return out.astype(np.float32)
